# revision 11
# baseline (speedup 1.0000x reference)
"""DeepseekV3 top-k router (moe_routing) on 8 Trainium2 NeuronCores.

Sharding (hardcoded from the problem spec):
  - Data-parallel over the token dim: 8192 tokens -> 8 shards of 1024.
  - Router weight [256, 7168] and bias [256] replicated to every core.

Numerics: logits = x.w need ~fp32 precision for the top-k indices to
match the fp32 reference exactly. Decompose x = xh + xl (fp16 hi +
residual), w = wh + wl, and compute
    logits = xh.wh  +  2^-18 * (fp8(xh).fp8(wl*2^18) + fp8(xl*2^12).fp8(wh*2^6))
with both correction terms in ONE fp8 DoubleRow matmul per k-tile
(DoubleRow contracts [K,2,M] x [K,2,N] over K and the 2-subtile dim).
Verified on the fixed eval inputs: 0/65536 index flips vs the fp32
reference, max weight rel err 1.3e-5.

Matmul orientation: the PE retires ~one matmul instruction per ~109ns
regardless of size (LDWEIGHTS is re-issued per matmul), so instructions
must carry >=109ns of work. With x as the moving operand (w stationary),
each instruction covers all 512 tokens of a superblock:
  - hh:  [K=128,128e].T @ [K=128,512t] fp16  -> 213ns work
  - corr:[K,2,128e].T @ [K,2,512t] fp8 DR    -> ~107ns work
  -> logits accumulate transposed [expert, token]; after combining
     hh+corr on ScalarE+DVE the 128x128 tiles are transposed back via
     PE-transpose against an identity (is_transpose matmul).
PE work drops from ~242ns to ~160ns per (128-token-tile, k-tile).

DMA: x ships as 3 B/elem (fp16 hi + fp8 lo); fp8(xh) is cast on-device
(DVE). w ships fp16 hi + two prescaled fp8 parts. 29.4 MB/core split
statically: sync ring carries xh (14.7 MB), scalar ring carries
xl8 + w16 + w8 (14.7 MB); every superblock chunked along k so matmuls
track chunk arrival. Epilogues are emitted one superblock late so the
next superblock's casts never queue behind them on the DVE.

Epilogue per 128-token tile (from the transposed PSUM): sigmoid on
ScalarE, DeepseekV3 grouped top-k on VectorE (group top-2 via segmented
max + match_replace, top-4 groups via sort8 + threshold, top-8 experts
via max/max_index, score gather via an 8x8 index match), *2.5.
"""

import os
import sys

for _p in ("/opt/trn_rl_repo", "/root/.axon_site/_ro/trn_rl_repo"):
    if os.path.isdir(_p) and _p not in sys.path:
        sys.path.append(_p)

from contextlib import ExitStack

import numpy as np
import ml_dtypes

import concourse.bass as bass
import concourse.bacc as bacc
import concourse.mybir as mybir
import concourse.tile as tile
from concourse import masks

N_CORES = 8
T_FULL = 8192
HIDDEN = 7168
N_EXPERTS = 256
TOP_K = 8
N_GROUP = 8
TOPK_GROUP = 4
SCALING = 2.5

P = 128
TB = 512                      # tokens per superblock (4 tiles)
F32 = mybir.dt.float32
F16 = mybir.dt.float16
F8 = mybir.dt.float8e4
E4NP = ml_dtypes.float8_e4m3
S_XL = 2.0 ** 12              # xl8 = fp8(xl * S_XL)
S_WL = 2.0 ** 18              # wl8 = fp8(wl * S_WL)
S_WH = 2.0 ** 6               # wh8 = fp8(wh * S_WH)
S_CORR = S_XL * S_WH          # both fp8 products land at this scale
WARMUP_MMS = 120


def build_module(t_shard=T_FULL // N_CORES, hidden=HIDDEN):
    """Build + compile the per-core Bass module (SPMD: same program, 8 cores)."""
    KT = hidden // P            # hidden k-tiles (56)
    TT = t_shard // P           # token tiles per core (8)
    NB = t_shard // TB          # superblocks (2)
    NS = TB // P                # token tiles per superblock (4)
    E = N_EXPERTS
    EPG = E // N_GROUP          # experts per group (32)
    AX = mybir.AxisListType
    OP = mybir.AluOpType
    DR = mybir.MatmulPerfMode.DoubleRow

    nc = bacc.Bacc("TRN2", debug=False, target_bir_lowering=False)

    xh_in = nc.dram_tensor("xh16", [NB, P, KT, TB], F16, kind="ExternalInput").ap()
    xl_in = nc.dram_tensor("xl8", [NB, P, KT, TB], F8, kind="ExternalInput").ap()
    w16_in = nc.dram_tensor("w16", [P, KT, E], F16, kind="ExternalInput").ap()
    w8_in = nc.dram_tensor("w8", [P, 2, KT, E], F8, kind="ExternalInput").ap()
    bias = nc.dram_tensor("bias", [E], F32, kind="ExternalInput").ap()
    out_i = nc.dram_tensor("topk_idx", [t_shard, TOP_K], mybir.dt.int32,
                           kind="ExternalOutput").ap()
    out_w = nc.dram_tensor("topk_w", [t_shard, TOP_K], F32,
                           kind="ExternalOutput").ap()
    sink = nc.dram_tensor("warm_sink", [P, 1], F32).ap()

    # k-chunk boundaries: small first chunk so the first matmuls start early
    cuts = [0, 4, 13, 22, 32, 44, KT]
    kranges = [(cuts[i], cuts[i + 1]) for i in range(len(cuts) - 1)]

    with tile.TileContext(nc) as tc, ExitStack() as ctx:
        const = ctx.enter_context(tc.tile_pool(name="const", bufs=1))
        wpool = ctx.enter_context(tc.tile_pool(name="wres", bufs=1))
        xpool = ctx.enter_context(tc.tile_pool(name="xin", bufs=1))
        cpool = ctx.enter_context(tc.tile_pool(name="cmb", bufs=1))
        spool = ctx.enter_context(tc.tile_pool(name="scr", bufs=2))
        smalls = ctx.enter_context(tc.tile_pool(name="small", bufs=2))
        opool = ctx.enter_context(tc.tile_pool(name="outs", bufs=1))
        pshh = ctx.enter_context(tc.tile_pool(name="pshh", bufs=1, space="PSUM"))
        pscc = ctx.enter_context(tc.tile_pool(name="pscc", bufs=1, space="PSUM"))
        ptp = ctx.enter_context(tc.tile_pool(name="ptp", bufs=2, space="PSUM"))
        pswarm = ctx.enter_context(tc.tile_pool(name="psw", bufs=1, space="PSUM"))

        # ---- PE warm-up: keep the HAM clock-gate busy from t=0 ----
        wu = const.tile([P, 64], F16)
        nc.gpsimd.memset(wu[:], 0.0)
        psw = pswarm.tile([P, 64], F32)
        for _ in range(WARMUP_MMS):
            nc.tensor.matmul(psw[:64], wu[:], wu[:], start=True, stop=True)
        wsum = smalls.tile([P, 1], F32, tag="wsum")
        nc.vector.tensor_reduce(wsum[:], psw[:], axis=AX.X, op=OP.add)
        # SWDGE ring: must not block the HWDGE rings while warmup runs
        nc.gpsimd.dma_start(out=sink, in_=wsum[:])

        # ---- constants ----
        bias_bc = const.tile([P, E], F32)
        bias_src = bass.AP(tensor=bias.tensor, offset=0, ap=[[0, P], [1, E]])
        ident = const.tile([P, P], F32)
        masks.make_identity(nc, ident[:])

        # ---- resident w: fp16 hi + fp8 (wl, wh) pair ----
        w16_sb = wpool.tile([P, KT, E], F16)
        w8_sb = wpool.tile([P, 2, KT, E], F8)

        out_i_sb = opool.tile([P, TT, TOP_K], mybir.dt.int32)
        out_w_sb = opool.tile([P, TT, TOP_K], F32)

        def epilogue_tile(tt, ps_t):
            # sigmoid scores from the transposed [token, expert] PSUM tile
            s = spool.tile([P, E], F32, tag="s")
            nc.scalar.activation(s[:], ps_t[:, :E],
                                 mybir.ActivationFunctionType.Sigmoid)

            # scores for choice = sigmoid + bias
            sc = spool.tile([P, E], F32, tag="sc")
            nc.vector.tensor_tensor(sc[:], s[:], bias_bc[:], op=OP.add)

            sc_g = sc[:].rearrange("p (g c) -> p g c", c=EPG)

            # per-group top-2 sum
            gmax = smalls.tile([P, N_GROUP], F32, tag="gmax")
            nc.vector.tensor_reduce(gmax[:], sc_g, axis=AX.X, op=OP.max)
            rep = spool.tile([P, E], F32, tag="rep")
            nc.vector.match_replace(rep[:], gmax[:], sc[:], -1e30)
            gsec = smalls.tile([P, N_GROUP], F32, tag="gsec")
            nc.vector.tensor_reduce(gsec[:],
                                    rep[:].rearrange("p (g c) -> p g c", c=EPG),
                                    axis=AX.X, op=OP.max)
            gsum = smalls.tile([P, N_GROUP], F32, tag="gsum")
            nc.vector.tensor_tensor(gsum[:], gmax[:], gsec[:], op=OP.add)

            # top-4 groups: sort the 8 group scores, threshold at 4th
            gsort = smalls.tile([P, 8], F32, tag="gsort")
            nc.vector.max(gsort[:], gsum[:])
            gmask = smalls.tile([P, N_GROUP], F32, tag="gmask")
            nc.vector.tensor_scalar(gmask[:], gsum[:],
                                    gsort[:, TOPK_GROUP - 1:TOPK_GROUP], None,
                                    op0=OP.is_ge)

            # masked scores = sc * group_mask
            masked = spool.tile([P, E], F32, tag="masked")
            nc.vector.tensor_tensor(masked[:].rearrange("p (g c) -> p g c", c=EPG),
                                    sc_g,
                                    gmask[:].unsqueeze(2).broadcast_to(
                                        (P, N_GROUP, EPG)),
                                    op=OP.mult)

            # top-8 experts (desc values + indices, lax.top_k semantics)
            t8v = smalls.tile([P, TOP_K], F32, tag="t8v")
            nc.vector.max(t8v[:], masked[:])
            t8i = smalls.tile([P, TOP_K], mybir.dt.uint32, tag="t8i")
            nc.vector.max_index(t8i[:], t8v[:], masked[:])

            # output copy rides GpSimd so it stays off the DVE chain
            nc.gpsimd.tensor_copy(out_i_sb[:, tt, :], t8i[:])

            # gather sigmoid scores at the top-8 indices
            mr2 = spool.tile([P, E], F32, tag="mr2")
            nc.vector.match_replace(mr2[:], t8v[:], masked[:], -1.0)
            sel = spool.tile([P, E], F32, tag="sel")
            nc.vector.tensor_tensor(sel[:], mr2[:], masked[:], op=OP.not_equal)
            nc.vector.tensor_tensor(sel[:], sel[:], s[:], op=OP.mult)
            v8 = smalls.tile([P, TOP_K], F32, tag="v8")
            nc.vector.max(v8[:], sel[:])
            i8 = smalls.tile([P, TOP_K], mybir.dt.uint32, tag="i8")
            nc.vector.max_index(i8[:], v8[:], sel[:])
            # eqm[p, k, j] = (idx_choice[p, k] == idx_s[p, j]); sg = eqm @ v8
            eqm = smalls.tile([P, TOP_K, TOP_K], F32, tag="eqm")
            nc.vector.tensor_tensor(eqm[:],
                                    t8i[:].unsqueeze(2).broadcast_to(
                                        (P, TOP_K, TOP_K)),
                                    i8[:].unsqueeze(1).broadcast_to(
                                        (P, TOP_K, TOP_K)),
                                    op=OP.is_equal)
            nc.vector.tensor_tensor(eqm[:], eqm[:],
                                    v8[:].unsqueeze(1).broadcast_to(
                                        (P, TOP_K, TOP_K)),
                                    op=OP.mult)
            sg = smalls.tile([P, TOP_K], F32, tag="sg")
            nc.vector.tensor_reduce(sg[:], eqm[:], axis=AX.X, op=OP.add)

            # weights = sg / sum(sg) * SCALING
            den = smalls.tile([P, 1], F32, tag="den")
            nc.vector.tensor_reduce(den[:], sg[:], axis=AX.X, op=OP.add)
            rcp = smalls.tile([P, 1], F32, tag="rcp")
            nc.vector.reciprocal(rcp[:], den[:])
            nc.vector.tensor_scalar(out_w_sb[:, tt, :], sg[:], rcp[:, 0:1],
                                    SCALING, op0=OP.mult, op1=OP.mult)

        nsub = TB // P
        oi = out_i.rearrange("(t p) k -> p t k", p=P)
        ow = out_w.rearrange("(t p) k -> p t k", p=P)

        def epilogue_sb(sb, ps_h, ps_c):
            # combine halves: stage hh via ScalarE, stt on DVE
            sA = cpool.tile([P, 2, TB], F32, tag="sA")
            comb = cpool.tile([P, 2, TB], F32, tag="comb")
            for h in range(2):
                nc.scalar.activation(sA[:, h], ps_h[:, h],
                                     mybir.ActivationFunctionType.Copy)
                nc.vector.scalar_tensor_tensor(comb[:, h], ps_c[:, h],
                                               1.0 / S_CORR, sA[:, h],
                                               op0=OP.mult, op1=OP.add)
            for tt in range(NS):
                ps_t = ptp.tile([P, E], F32, tag="pst")
                for h in range(2):
                    nc.tensor.transpose(ps_t[:, h * P:(h + 1) * P],
                                        comb[:, h, tt * P:(tt + 1) * P],
                                        ident[:])
                epilogue_tile(sb * NS + tt, ps_t)
            t0 = sb * NS
            nc.scalar.dma_start(out=oi[:, t0:t0 + NS],
                                in_=out_i_sb[:, t0:t0 + NS])
            nc.scalar.dma_start(out=ow[:, t0:t0 + NS],
                                in_=out_w_sb[:, t0:t0 + NS])

        pending = None
        for sb in range(NB):
            xh_t = xpool.tile([P, KT, TB], F16, tag="xh", name=f"xh_{sb}")
            x8_t = xpool.tile([P, 2, KT, TB], F8, tag="x8", name=f"x8_{sb}")
            for c, (k0, k1) in enumerate(kranges):
                nc.sync.dma_start(out=xh_t[:, k0:k1], in_=xh_in[sb, :, k0:k1])
                nc.scalar.dma_start(out=x8_t[:, 1, k0:k1],
                                    in_=xl_in[sb, :, k0:k1])
                if sb == 0:
                    nc.scalar.dma_start(out=w16_sb[:, k0:k1],
                                        in_=w16_in[:, k0:k1])
                    nc.scalar.dma_start(out=w8_sb[:, :, k0:k1],
                                        in_=w8_in[:, :, k0:k1])
                # on-device cast x8[:,0] = fp8(xh) on the DVE
                nc.vector.tensor_copy(x8_t[:, 0, k0:k1], xh_t[:, k0:k1])
            if sb == 0:
                nc.scalar.dma_start(out=bias_bc[:], in_=bias_src)

            ps_h = pshh.tile([P, 2, TB], F32, tag="psh")
            ps_c = pscc.tile([P, 2, TB], F32, tag="psc")
            # per k-chunk: all hh matmuls, then all DR matmuls (mode flips
            # only twice per chunk; runs keep the PE streaming)
            for (k0, k1) in kranges:
                for k in range(k0, k1):
                    for h in range(2):
                        nc.tensor.matmul(ps_h[:, h],
                                         w16_sb[:, k, h * P:(h + 1) * P],
                                         xh_t[:, k],
                                         start=(k == 0), stop=(k == KT - 1))
                for k in range(k0, k1):
                    for h in range(2):
                        nc.tensor.matmul(ps_c[:, h],
                                         w8_sb[:, :, k, h * P:(h + 1) * P],
                                         x8_t[:, :, k],
                                         start=(k == 0), stop=(k == KT - 1),
                                         perf_mode=DR)

            # emit the PREVIOUS superblock's epilogue now, so this one's
            # casts did not queue behind it on the DVE
            if pending is not None:
                epilogue_sb(*pending)
            pending = (sb, ps_h, ps_c)
        epilogue_sb(*pending)

    nc.compile()
    return nc


_CACHED = {}


def _get_module():
    key = (T_FULL // N_CORES, HIDDEN)
    if key not in _CACHED:
        _CACHED[key] = build_module(*key)
    return _CACHED[key]


def _tile_x(shardT, t_shard, hidden):
    # [H, T] -> [NB, P, KT, TB]   (h = k*P + p, t = nb*TB + c)
    KT = hidden // P
    NB = t_shard // TB
    v = shardT.reshape(KT, P, NB, TB)
    return np.ascontiguousarray(v.transpose(2, 1, 0, 3))


def _make_in_maps(x, weight, e_score_correction_bias):
    x = np.asarray(x, dtype=np.float32)
    w = np.asarray(weight, dtype=np.float32)
    b = np.ascontiguousarray(np.asarray(e_score_correction_bias, dtype=np.float32))
    hidden = x.shape[1]
    E = w.shape[0]
    KT = hidden // P

    wT = np.ascontiguousarray(w.T)                      # [H, E] f32
    w16 = wT.astype(np.float16)
    wl8 = ((wT - w16.astype(np.float32)) * np.float32(S_WL)).astype(E4NP)
    wh8 = (w16.astype(np.float32) * np.float32(S_WH)).astype(E4NP)

    def tile_w(a):                                      # [H, E] -> [P, KT, E]
        return np.ascontiguousarray(a.reshape(KT, P, E).transpose(1, 0, 2))

    w16_t = tile_w(w16)
    w8_t = np.ascontiguousarray(
        np.stack([tile_w(wl8), tile_w(wh8)], axis=1))   # [P, 2, KT, E]

    t_shard = x.shape[0] // N_CORES
    in_maps = []
    for i in range(N_CORES):
        shardT = np.ascontiguousarray(x[i * t_shard:(i + 1) * t_shard].T)
        xh = shardT.astype(np.float16)
        xl8 = ((shardT - xh.astype(np.float32))
               * np.float32(S_XL)).astype(E4NP)
        in_maps.append({"xh16": _tile_x(xh, t_shard, hidden),
                        "xl8": _tile_x(xl8, t_shard, hidden),
                        "w16": w16_t, "w8": w8_t, "bias": b})
    return in_maps


def run_hw(x, weight, e_score_correction_bias, trace=False, **kwargs):
    """Run on the 8 NeuronCores; returns ((idx, w), BassKernelResults)."""
    from concourse.bass_utils import run_bass_kernel_spmd

    nc = _get_module()
    in_maps = _make_in_maps(x, weight, e_score_correction_bias)
    res = run_bass_kernel_spmd(nc, in_maps, core_ids=list(range(N_CORES)),
                               trace=trace, **kwargs)
    idx = np.concatenate([r["topk_idx"] for r in res.results], axis=0)
    w = np.concatenate([r["topk_w"] for r in res.results], axis=0)
    return (idx.astype(np.int32, copy=False), w.astype(np.float32, copy=False)), res


def kernel(x, weight, e_score_correction_bias):
    (idx, w), _ = run_hw(x, weight, e_score_correction_bias, trace=False)
    return idx, w
